# revision 42
# baseline (speedup 1.0000x reference)
"""BatchAllTripletLoss kernel for 8 Trainium2 NeuronCores.

Reference computation:
    pd = pairwise_euclidean(rep)                        # [512, 512]
    tl[a,p,k] = relu(pd[a,p] - pd[a,k] + 5.0) * mask    # [512, 512, 512]
    loss = sum(tl) / (count(tl > eps) + eps)

Valid triplets are (anchor-positive pairs) x (k with a different label):
with 64 labels over 512 rows there are ~3930 (a,p) pairs. Anchors are
partitioned into 8 groups of exactly 64, chosen so per-core pair counts
balance to <=512 (4 tiles of 128 pairs). Per core the columns are
permuted so each tile's positive columns occupy one 128-column block;
the 64 anchor embeddings ride as duplicated columns 512:576 of rept.

Pipeline (redesign of the proven baseline, same math):
  The 256-dim dot rides ONE fp8-e4m3 DoubleRow matmul per column half
  (K=256 packed two-per-partition, 0.5 cycles/row); the sq_j row is
  computed host-side from the quantized rep and rides a K=1 fp16 pass
  (ones x (-sq_j/2)); sq_a rides the sqrt's per-partition ACT bias
  (host column sq_a+1).  d = sqrt(-2*(dot - sq_j/2) + sq_a + 1), split
  in column halves so sqrt h1 starts while PE does h2.  Per pair tile
  t: a redundant 128-col window matmul (stationary selX whose label
  rows carry margin instead of B) into its own PSUM bank feeds the DVE
  extraction xp = d_ap + margin directly; ACT relu(xp - gy) accumulates
  S; counts ride DVE from the bf16 relu output.

The same-label mask rides inside the gather matmul (stationary rows
64:128 = B*onehot(label(anchor)), moving partitions 64:128 = the label
indicator rows Lk), so gy[k] = d_ak + B*same(a,k).  B = 64 kills masked
k in both relu and count.  The +1 inside sqrt keeps the (masked)
diagonal's rounding noise out of sqrt's domain; its effect on
d_ap - d_ak cancels to ~1e-4.

DMA layout (measured, not guessed): rept leads the SP HWDGE queue as a
single 1152B-per-partition fp8 DMA - big descriptors drain fastest and
a DMA's 16 completion markers are delayed by descriptors queued close
behind it, so the two sel tensors follow rept on the same queue while
the ACT queue carries only the tiny sqj/aux2 rows.  Two dummy
activations are emitted before the DMAs so walrus's ACT_TABLE_LOADs
(2x 1283ns) run during the DMA wait, not before the first real sqrt.
The 8 cores' per-partition partial sums/counts [128, 2*Tp] are reduced
on the host (the all-reduce of the sharding hint).  Host-side prep is
layout/mask logic plus the sq row of the quantized rep.

Exit protocol: bass semaphores are allocated from [207,256) - the range
the runtime's end-of-NEFF sweep assigns to the SYNC engine - and the
tile exit emits ONLY a SYNC drain that waits out the full tile clock.
Every other engine's stream ends at its last real instruction, so the
runtime's fixed ~250-semaphore zeroing sweep (~6.9us) starts as soon as
the SYNC drain + output DMA complete.
"""

import ml_dtypes
import numpy as np

import concourse.bass as bass
import concourse.bass_utils as bass_utils
import concourse.tile as tile
from concourse import bacc, mybir
from concourse.bass_utils import run_bass_kernel_spmd
from concourse.vector_clock import ScopedClock

# (probed: walrus --max-sem-num does NOT shrink the fixed end-of-NEFF
# 256-semaphore sweep, so the epilogue cost is invariant; keep default)

F32 = mybir.dt.float32
BF16 = mybir.dt.bfloat16
FP16 = mybir.dt.float16
FP8 = mybir.dt.float8e4
AF = mybir.ActivationFunctionType
OP = mybir.AluOpType

N = 512          # rows
D = 256          # embedding dim
NCORES = 8
A = N // NCORES  # anchors per core
NLAB = 64        # label values
MARGIN = 5.0
EPS = 1e-16
BIGB = 64.0      # same-label mask bias (power of two)
XOFF = MARGIN - BIGB

_orig_aeb = bass.Bass.all_engine_barrier
_orig_sem_range = bass.get_kernel_semaphore_range


def _skip_const_barrier(self, *, sem_only=False):
    # The runtime prologue already barriers all engines before bass code.
    if not getattr(self, "_aeb_skipped_once", False):
        self._aeb_skipped_once = True
        return
    return _orig_aeb(self, sem_only=sem_only)


SAFE_EXIT = False


def _safe_exit(self, tick_clock, wait_clock):
    """Baseline exit: SP drain waits the tile clock, then sem cleanup and
    sequencer-only barriers (proven on hardware)."""
    drain_inst = self.nc.sync.drain()
    wait_clock.add_sem_waits(
        drain_inst.ins, ScopedClock({None: tick_clock.global_clock})
    )
    self.nc.all_engine_barrier(sem_only=True)
    popped = self.nc._tile_sem_poison_stack.pop()
    assert popped is self._sem_poison
    self.nc.clear_and_free_semaphores(list(self.sems.allocated().values()))
    self.nc.all_engine_barrier(sem_only=True)


def _sync_only_exit(self, tick_clock, wait_clock):
    """Exit protocol: SYNC drain waits the full tile clock.  No all-engine
    barrier and no semaphore clears: the runtime end-of-NEFF sweep zeroes
    everything, and bass sems live in SYNC's sweep range (207-255), which
    runs strictly after the full drain.  Every other engine's stream ends
    at its last real instruction."""
    drain_inst = self.nc.sync.drain()
    wait_clock.add_sem_waits(
        drain_inst.ins, ScopedClock({None: tick_clock.global_clock})
    )
    popped = self.nc._tile_sem_poison_stack.pop()
    assert popped is self._sem_poison
    sem_nums = [s.num for s in self.sems.allocated().values()]
    self.nc._state.prepend_free_semaphores(sem_nums)
    for poison_set in self.nc._tile_sem_poison_stack:
        poison_set.update(sem_nums)


_cache = {}


def _build(Tp: int):
    """Build the (uniform, SPMD) per-core Bass program for Tp pair tiles."""
    tile.TileContext._drain_and_barrier = (
        _safe_exit if SAFE_EXIT else _sync_only_exit)
    bass.Bass.all_engine_barrier = _skip_const_barrier
    if not SAFE_EXIT:
        bass.get_kernel_semaphore_range = lambda: range(207, 256)
    try:
        nc = bacc.Bacc(None, target_bir_lowering=False, num_swdge_queues=2)
    finally:
        bass.get_kernel_semaphore_range = _orig_sem_range

    # rept: cols 0:512 permuted rows x_j, cols 512:576 the 64 anchors again
    rept_d = nc.declare_dram_parameter("rept", [128, 2, N + A], FP8, isOutput=False)
    sel_d = nc.declare_dram_parameter("sel", [128, Tp * 128], FP16, isOutput=False)
    selx_d = nc.declare_dram_parameter("selx", [128, Tp * 128], FP16, isOutput=False)
    lk_d = nc.declare_dram_parameter("lk", [NLAB, N], FP16, isOutput=False)
    sqj_d = nc.declare_dram_parameter("sqj", [1, N], FP16, isOutput=False)  # -sq_j/2
    # aux2: cols 0:Tp pidx (global window col per pair slot), col Tp rows
    # 0:64 = sq_a + 1 (the sqrt bias)
    aux2_d = nc.declare_dram_parameter("aux2", [128, Tp + 1], F32, isOutput=False)
    out_d = nc.declare_dram_parameter("out", [128, 2 * Tp], F32, isOutput=True)
    import os
    dbg = os.environ.get("KDBG")
    if dbg:
        dbgM_d = nc.declare_dram_parameter("dbgM", [128, N], FP16, isOutput=True)
        dbgx_d = nc.declare_dram_parameter("dbgx", [128, 2 * Tp], F32, isOutput=True)

    out_sem = nc.alloc_semaphore("out_dma_sem")
    with tile.TileContext(nc) as tc:
        with (
            tc.tile_pool(name="singles", bufs=1) as sg,
            tc.tile_pool(name="ppda", bufs=1, space="PSUM") as ppda,
            tc.tile_pool(name="ppdb", bufs=1, space="PSUM") as ppdb,
            tc.tile_pool(name="ppg", bufs=4, space="PSUM") as ppg,
            tc.tile_pool(name="ppwa", bufs=1, space="PSUM") as ppwa,
            tc.tile_pool(name="ppwb", bufs=1, space="PSUM") as ppwb,
        ):
            # ---- input DMAs first.  The two HWDGE queues (SP + ACT) carry
            # a rept half each, then the tiny sqj/aux2 rows, then a sel
            # half each; only the Lk block (needed latest) rides the pool
            # SWDGE queue.
            rept_s = sg.tile([128, 2, N + A], FP8)
            sel_s = sg.tile([128, Tp * 128], FP16)
            selx_s = sg.tile([128, Tp * 128], FP16)
            sqj_s = sg.tile([1, N], FP16)
            aux2_s = sg.tile([128, Tp + 1], F32)
            M = sg.tile([128, N], FP16)   # rows 0:64 d', rows 64:128 Lk
            # dummy activations FIRST: walrus inserts each ACT_TABLE_LOAD
            # (1283ns, on the ACT engine) right before the first ACTIVATE
            # that needs its table, so warming both tables here runs the
            # loads during the DMA wait instead of delaying the first sqrt.
            ones1 = sg.tile([1, A], FP16)
            nc.vector.memset(ones1[:], 1.0)
            onef = sg.tile([1, 1], F32)
            nc.vector.memset(onef[:], 1.0)
            dmy = sg.tile([1, 1], F32)
            nc.scalar.activation(dmy[:], onef[:], AF.Sqrt, bias=onef[:],
                                 scale=onef[:])
            nc.scalar.activation(dmy[:], onef[:], AF.Relu, bias=onef[:],
                                 scale=-1.0)

            # rept leads the SP queue as one 1152B-per-partition DMA (big
            # descriptors drain fastest), selx/sel follow it there; the ACT
            # queue carries only the tiny sqj/aux2 so its sequencer frees
            # for the dummy activations while the tables load; Lk on pool.
            nc.sync.dma_start(rept_s[:], rept_d[:])
            nc.sync.dma_start(aux2_s[:], aux2_d[:])
            nc.sync.dma_start(selx_s[:], selx_d[:])
            nc.sync.dma_start(sel_s[:], sel_d[:])
            nc.scalar.dma_start(sqj_s[:], sqj_d[:])
            nc.gpsimd.dma_start(M[NLAB:128, :], lk_d[:])
            pidx_s = aux2_s[:, 0:Tp]
            bias_s = aux2_s[0:A, Tp:Tp + 1]

            # ---- constants / one-offs that overlap the DMA wait
            iota_f = sg.tile([128, N], F32)
            nc.gpsimd.iota(
                iota_f[:], [[1, N]], channel_multiplier=0,
                allow_small_or_imprecise_dtypes=True,
            )


            # ---- d2 accumulation: P = dot - sq_j/2 in column halves, one
            # PSUM bank per half so each bank sees a single clean
            # accumulation group (interleaved start-flags on one bank wipe
            # each other).  aug pass first (its row lands early over
            # SWDGE); h1 stops before h2 so sqrt h1 overlaps PE h2.
            d2a = ppda.tile([A, 256], F32, tag="d2a", name="d2a")
            d2b = ppdb.tile([A, 256], F32, tag="d2b", name="d2b")
            gys = [ppg.tile([128, N], F32, tag="gy", name=f"gy{t}")
                   for t in range(Tp)]
            # d2 groups: one fp8 DoubleRow pass fuses both 128-dim chunks
            # (K=256 packed two-per-partition at 0.5 cycles/row); the aug
            # (sqj) pass comes last so PE never stalls on it
            DR = mybir.MatmulPerfMode.DoubleRow
            nc.tensor.matmul(d2a[:], rept_s[:, :, N:], rept_s[:, :, 0:256],
                             start=True, stop=False, perf_mode=DR,
                             skip_group_check=True)
            nc.tensor.matmul(d2a[:], ones1[:], sqj_s[0:1, 0:256],
                             start=False, stop=True, skip_group_check=True)
            nc.tensor.matmul(d2b[:], rept_s[:, :, N:], rept_s[:, :, 256:512],
                             start=True, stop=False, perf_mode=DR,
                             skip_group_check=True)
            nc.tensor.matmul(d2b[:], ones1[:], sqj_s[0:1, 256:512],
                             start=False, stop=True, skip_group_check=True)

            # d' = sqrt(-2*P + (sq_a + 1)) into M rows 0:64, per half
            nc.scalar.activation(M[0:A, 0:256], d2a[:], AF.Sqrt,
                                 bias=bias_s, scale=-2.0)
            nc.scalar.activation(M[0:A, 256:512], d2b[:], AF.Sqrt,
                                 bias=bias_s, scale=-2.0)

            # ---- gathers.  Each tile's 128-col extraction window rides a
            # redundant matmul into a separate PSUM bank (two windows per
            # bank) so the DVE extraction never waits on a full-width gy.
            # gy0 is split in column halves to start right after sqrt h1.
            SC = sg.tile([128, 2 * Tp], F32)
            relbig = sg.tile([128, Tp, N], BF16)
            cscr = sg.tile([128, N], BF16)
            xscr = sg.tile([128, 2, 128], F32)
            xv = sg.tile([128, Tp], F32)
            winA = ppwa.tile([128, 2, 128], F32, tag="winA")
            winB = ppwb.tile([128, 2, 128], F32, tag="winB")
            wins = [winA[:, 0, :], winA[:, 1, :], winB[:, 0, :], winB[:, 1, :]]

            def selt(t):
                return sel_s[:, t * 128:(t + 1) * 128]

            def selxt(t):
                return selx_s[:, t * 128:(t + 1) * 128]

            # the windows use selX (label rows scaled to B+XOFF = margin),
            # so the extraction yields xp = d_ap + margin directly
            nc.tensor.matmul(wins[0], selxt(0), M[:, 0:128],
                             start=True, stop=True)
            nc.tensor.matmul(gys[0][:, 0:256], selt(0), M[:, 0:256],
                             start=True, stop=True)
            nc.tensor.matmul(wins[1], selxt(1), M[:, 128:256],
                             start=True, stop=True)
            nc.tensor.matmul(gys[0][:, 256:512], selt(0), M[:, 256:512],
                             start=True, stop=True)
            nc.tensor.matmul(wins[2], selxt(2), M[:, 256:384],
                             start=True, stop=True)
            nc.tensor.matmul(wins[3], selxt(3), M[:, 384:512],
                             start=True, stop=True)
            for t in range(1, Tp):
                nc.tensor.matmul(gys[t][:], selt(t), M[:],
                                 start=True, stop=True)

            # ---- extraction (DVE): xv[p] = d_ap + margin from the window
            def extract(t):
                w0 = t * 128
                nc.vector.scalar_tensor_tensor(
                    out=xscr[:, t % 2, :], in0=iota_f[:, w0:w0 + 128],
                    scalar=pidx_s[:, t:t + 1], in1=wins[t],
                    op0=OP.is_equal, op1=OP.mult, accum_out=xv[:, t:t + 1],
                )

            for t in range(Tp):
                extract(t)

            # ---- relu + S on ACT; counts on DVE from the bf16 relu output
            for t in range(Tp):
                nc.scalar.activation(
                    relbig[:, t, :], gys[t][:], AF.Relu, bias=xv[:, t:t + 1],
                    scale=-1.0, accum_out=SC[:, t:t + 1],
                )
            for t in range(Tp):
                nc.vector.tensor_scalar(
                    cscr[:], relbig[:, t, :], 0.0, 0.0,
                    OP.is_gt, OP.add, accum_out=SC[:, Tp + t:Tp + t + 1],
                )

            # pool's stream must outlive its own SWDGE input DMA (Lk): a
            # tiny read makes the tile tracker emit the wait.
            pguard = sg.tile([1, 8], FP16)
            nc.gpsimd.tensor_copy(pguard[0:1, 0:8], M[NLAB:NLAB + 1, 0:8])

    # Output DMA emitted AFTER the tile exit: SP executes it post-drain
    # (every compute tick already waited out, so SC is final) and nothing
    # waits on its completion - it flies during the runtime's semaphore
    # sweep.  The completion sem is required plumbing; nothing waits on it.
    out_insts = [nc.sync.dma_start(out_d[:], SC[:]).then_inc(out_sem, 16)]
    if dbg:
        out_insts.append(
            nc.sync.dma_start(dbgM_d[:], M[:]).then_inc(out_sem, 16))
        xvxp = sg_xvxp = None
        out_insts.append(
            nc.sync.dma_start(dbgx_d[:, 0:Tp], xv[:]).then_inc(out_sem, 16))

    def _concrete(arg):
        t = arg.bass_ap.tensor
        if hasattr(t, "concrete_tensor"):
            try:
                arg.bass_ap.tensor = t.concrete_tensor()
            except Exception:
                pass
        return arg.bass_ap

    for oi in out_insts:
        raw = oi.ins
        raw.ins, raw.outs = nc.sync.lower_symbolic_args(
            raw.ins, raw.outs, _concrete, raw.debug)

    nc.finalize()
    return nc


def _prep(rep: np.ndarray, labels: np.ndarray):
    """Host-side prep: shard anchors, enumerate pairs, build layouts."""
    rep = np.ascontiguousarray(np.asarray(rep, dtype=np.float32))
    labels = np.asarray(labels).astype(np.int64)
    cnt = np.bincount(labels, minlength=NLAB)
    ppa = cnt[labels] - 1              # pairs per anchor
    rows_of = [np.nonzero(labels == l)[0] for l in range(NLAB)]

    # balance pair counts across 8 groups of exactly 64 anchors
    order = np.argsort(-ppa, kind="stable")
    groups = [[] for _ in range(NCORES)]
    loads = [0] * NCORES
    for a in order:
        cand = min((i for i in range(NCORES) if len(groups[i]) < A),
                   key=lambda j: loads[j])
        groups[cand].append(int(a))
        loads[cand] += int(ppa[a])
    Tp = max(1, (max(loads) + 127) // 128)

    rep8 = rep.astype(ml_dtypes.float8_e4m3fn)
    sqj_all = (rep8.astype(np.float32) ** 2).sum(axis=1)   # from quantized rep
    in_maps = []
    for c in range(NCORES):
        anchors = groups[c]
        # bin-pack the 512 columns into Tp blocks of 128 so each block's
        # positive-pair count fits one 128-pair tile
        anchset = np.zeros(N, bool)
        anchset[anchors] = True
        m = np.zeros(N, np.int64)
        for p in range(N):
            m[p] = sum(1 for a in rows_of[labels[p]]
                       if anchset[a] and a != p)
        blocks = [[] for _ in range(Tp)]
        bsum = [0] * Tp
        for p in np.argsort(-m, kind="stable"):
            cand = [b for b in range(Tp)
                    if len(blocks[b]) < 128 and bsum[b] + m[p] <= 128]
            b = min(cand, key=lambda x: bsum[x])
            blocks[b].append(int(p))
            bsum[b] += int(m[p])
        perm = np.array([p for b in blocks for p in b], np.int64)
        colof = np.empty(N, np.int64)
        colof[perm] = np.arange(N)
        aidx = {int(a): i for i, a in enumerate(anchors)}

        repp = rep8[perm]                                  # [512, 256]
        repe = np.concatenate([repp, rep8[anchors]])       # + anchor dup
        rept = np.ascontiguousarray(
            repe.T.reshape(2, 128, N + A).transpose(1, 0, 2)
        )
        sel = np.zeros((128, Tp * 128), np.float16)
        lk = np.ascontiguousarray(
            (labels[perm][None, :] == np.arange(NLAB)[:, None])
            .astype(np.float16))
        sqj = np.ascontiguousarray(
            (-0.5 * sqj_all[perm])[None, :].astype(np.float16))
        aux2 = np.zeros((128, Tp + 1), np.float32)
        aux2[0:A, Tp] = sqj_all[anchors] + 1.0
        for t in range(Tp):
            aux2[:, t] = 128 * t    # pad pidx inside window -> xv = 0
        nt = [0] * Tp
        for ga in anchors:
            a = aidx[ga]
            for p in rows_of[labels[ga]]:
                if p == ga:
                    continue
                t, r = colof[p] // 128, nt[colof[p] // 128]
                nt[t] += 1
                sel[a, t * 128 + r] = 1.0
                sel[A + labels[ga], t * 128 + r] = BIGB
                aux2[r, t] = colof[p]
        selx = sel.copy()
        selx[A:2 * A] *= (BIGB + XOFF) / BIGB   # label rows -> margin
        in_maps.append({"rept": rept, "sel": sel, "selx": selx, "lk": lk,
                        "sqj": sqj, "aux2": aux2})
    return Tp, in_maps


def _run(rep, labels, trace=False):
    Tp, in_maps = _prep(rep, labels)
    if Tp not in _cache:
        _cache[Tp] = _build(Tp)
    nc = _cache[Tp]
    res = run_bass_kernel_spmd(nc, in_maps, list(range(NCORES)), trace=trace)
    outs = np.stack([res.results[c]["out"] for c in range(NCORES)])  # [8,128,2Tp]
    S = float(outs[:, :, :Tp].sum(dtype=np.float64))
    C = float(outs[:, :, Tp:].sum(dtype=np.float64))
    loss = np.float32(S / (C + EPS))
    return np.asarray(loss, dtype=np.float32), res


def kernel(rep, labels):
    loss, _ = _run(rep, labels, trace=False)
    return loss


# revision 43
# speedup vs baseline: 1.0419x; 1.0419x over previous
"""BatchAllTripletLoss kernel for 8 Trainium2 NeuronCores.

Reference computation:
    pd = pairwise_euclidean(rep)                        # [512, 512]
    tl[a,p,k] = relu(pd[a,p] - pd[a,k] + 5.0) * mask    # [512, 512, 512]
    loss = sum(tl) / (count(tl > eps) + eps)

Valid triplets are (anchor-positive pairs) x (k with a different label):
with 64 labels over 512 rows there are ~3930 (a,p) pairs. Anchors are
partitioned into 8 groups of exactly 64, chosen so per-core pair counts
balance to <=512 (4 tiles of 128 pairs). Per core the columns are
permuted so each tile's positive columns occupy one 128-column block;
the 64 anchor embeddings ride as duplicated columns 512:576 of rept.

Pipeline (redesign of the proven baseline, same math):
  The 256-dim dot rides ONE fp8-e4m3 DoubleRow matmul per column half
  (K=256 packed two-per-partition, 0.5 cycles/row); the sq_j row is
  computed host-side from the quantized rep and rides a K=1 fp16 pass
  (ones x (-sq_j/2)); sq_a rides the sqrt's per-partition ACT bias
  (host column sq_a+1).  d = sqrt(-2*(dot - sq_j/2) + sq_a + 1), split
  in column halves so sqrt h1 starts while PE does h2.  Per pair tile
  t: a redundant 128-col window matmul (stationary selX whose label
  rows carry margin instead of B) into its own PSUM bank feeds the DVE
  extraction xp = d_ap + margin directly; ACT relu(xp - gy) accumulates
  S; counts ride DVE from the bf16 relu output.

The same-label mask rides inside the gather matmul (stationary rows
64:128 = B*onehot(label(anchor)), moving partitions 64:128 = the label
indicator rows Lk), so gy[k] = d_ak + B*same(a,k).  B = 64 kills masked
k in both relu and count.  The +1 inside sqrt keeps the (masked)
diagonal's rounding noise out of sqrt's domain; its effect on
d_ap - d_ak cancels to ~1e-4.

DMA layout (measured, not guessed): rept leads the SP HWDGE queue as a
single 1152B-per-partition fp8 DMA - big descriptors drain fastest and
a DMA's 16 completion markers are delayed by descriptors queued close
behind it, so the two sel tensors follow rept on the same queue while
the ACT queue carries only the tiny sqj/aux2 rows.  Two dummy
activations are emitted before the DMAs so walrus's ACT_TABLE_LOADs
(2x 1283ns) run during the DMA wait, not before the first real sqrt.
The 8 cores' per-partition partial sums/counts [128, 2*Tp] are reduced
on the host (the all-reduce of the sharding hint).  Host-side prep is
layout/mask logic plus the sq row of the quantized rep.

Exit protocol: bass semaphores are allocated from [207,256) - the range
the runtime's end-of-NEFF sweep assigns to the SYNC engine - and the
tile exit emits ONLY a SYNC drain that waits out the full tile clock.
Every other engine's stream ends at its last real instruction, so the
runtime's fixed ~250-semaphore zeroing sweep (~6.9us) starts as soon as
the SYNC drain + output DMA complete.
"""

import ml_dtypes
import numpy as np

import concourse.bass as bass
import concourse.bass_utils as bass_utils
import concourse.tile as tile
from concourse import bacc, mybir
from concourse.bass_utils import run_bass_kernel_spmd
from concourse.vector_clock import ScopedClock

# (probed: walrus --max-sem-num does NOT shrink the fixed end-of-NEFF
# 256-semaphore sweep, so the epilogue cost is invariant; keep default)

F32 = mybir.dt.float32
BF16 = mybir.dt.bfloat16
FP16 = mybir.dt.float16
FP8 = mybir.dt.float8e4
AF = mybir.ActivationFunctionType
OP = mybir.AluOpType

N = 512          # rows
D = 256          # embedding dim
NCORES = 8
A = N // NCORES  # anchors per core
NLAB = 64        # label values
MARGIN = 5.0
EPS = 1e-16
BIGB = 64.0      # same-label mask bias (power of two)
XOFF = MARGIN - BIGB

_orig_aeb = bass.Bass.all_engine_barrier
_orig_sem_range = bass.get_kernel_semaphore_range


def _skip_const_barrier(self, *, sem_only=False):
    # The runtime prologue already barriers all engines before bass code.
    if not getattr(self, "_aeb_skipped_once", False):
        self._aeb_skipped_once = True
        return
    return _orig_aeb(self, sem_only=sem_only)


SAFE_EXIT = False


def _safe_exit(self, tick_clock, wait_clock):
    """Baseline exit: SP drain waits the tile clock, then sem cleanup and
    sequencer-only barriers (proven on hardware)."""
    drain_inst = self.nc.sync.drain()
    wait_clock.add_sem_waits(
        drain_inst.ins, ScopedClock({None: tick_clock.global_clock})
    )
    self.nc.all_engine_barrier(sem_only=True)
    popped = self.nc._tile_sem_poison_stack.pop()
    assert popped is self._sem_poison
    self.nc.clear_and_free_semaphores(list(self.sems.allocated().values()))
    self.nc.all_engine_barrier(sem_only=True)


def _sync_only_exit(self, tick_clock, wait_clock):
    """Exit protocol: SYNC drain waits the full tile clock.  No all-engine
    barrier and no semaphore clears: the runtime end-of-NEFF sweep zeroes
    everything, and bass sems live in SYNC's sweep range (207-255), which
    runs strictly after the full drain.  Every other engine's stream ends
    at its last real instruction."""
    drain_inst = self.nc.sync.drain()
    wait_clock.add_sem_waits(
        drain_inst.ins, ScopedClock({None: tick_clock.global_clock})
    )
    popped = self.nc._tile_sem_poison_stack.pop()
    assert popped is self._sem_poison
    sem_nums = [s.num for s in self.sems.allocated().values()]
    self.nc._state.prepend_free_semaphores(sem_nums)
    for poison_set in self.nc._tile_sem_poison_stack:
        poison_set.update(sem_nums)


_cache = {}


def _build(Tp: int):
    """Build the (uniform, SPMD) per-core Bass program for Tp pair tiles."""
    tile.TileContext._drain_and_barrier = (
        _safe_exit if SAFE_EXIT else _sync_only_exit)
    bass.Bass.all_engine_barrier = _skip_const_barrier
    if not SAFE_EXIT:
        bass.get_kernel_semaphore_range = lambda: range(207, 256)
    try:
        nc = bacc.Bacc(None, target_bir_lowering=False, num_swdge_queues=2)
    finally:
        bass.get_kernel_semaphore_range = _orig_sem_range

    # rept: cols 0:512 permuted rows x_j, cols 512:576 the 64 anchors again
    rept_d = nc.declare_dram_parameter("rept", [128, 2, N + A], FP8, isOutput=False)
    sel_d = nc.declare_dram_parameter("sel", [128, Tp * 128], FP16, isOutput=False)
    selx_d = nc.declare_dram_parameter("selx", [128, Tp * 128], FP16, isOutput=False)
    lk_d = nc.declare_dram_parameter("lk", [NLAB, N], FP16, isOutput=False)
    sqj_d = nc.declare_dram_parameter("sqj", [1, N], FP16, isOutput=False)  # -sq_j/2
    # aux2: cols 0:Tp pidx (global window col per pair slot), col Tp rows
    # 0:64 = sq_a + 1 (the sqrt bias)
    aux2_d = nc.declare_dram_parameter("aux2", [128, Tp + 1], F32, isOutput=False)
    out_d = nc.declare_dram_parameter("out", [128, 2 * Tp], F32, isOutput=True)
    import os
    dbg = os.environ.get("KDBG")
    if dbg:
        dbgM_d = nc.declare_dram_parameter("dbgM", [128, N], FP16, isOutput=True)
        dbgx_d = nc.declare_dram_parameter("dbgx", [128, 2 * Tp], F32, isOutput=True)

    out_sem = nc.alloc_semaphore("out_dma_sem")
    with tile.TileContext(nc) as tc:
        with (
            tc.tile_pool(name="singles", bufs=1) as sg,
            tc.tile_pool(name="ppda", bufs=1, space="PSUM") as ppda,
            tc.tile_pool(name="ppdb", bufs=1, space="PSUM") as ppdb,
            tc.tile_pool(name="ppg", bufs=4, space="PSUM") as ppg,
            tc.tile_pool(name="ppwa", bufs=1, space="PSUM") as ppwa,
            tc.tile_pool(name="ppwb", bufs=1, space="PSUM") as ppwb,
        ):
            # ---- input DMAs first.  The two HWDGE queues (SP + ACT) carry
            # a rept half each, then the tiny sqj/aux2 rows, then a sel
            # half each; only the Lk block (needed latest) rides the pool
            # SWDGE queue.
            rept_s = sg.tile([128, 2, N + A], FP8)
            sel_s = sg.tile([128, Tp * 128], FP16)
            selx_s = sg.tile([128, Tp * 128], FP16)
            sqj_s = sg.tile([1, N], FP16)
            aux2_s = sg.tile([128, Tp + 1], F32)
            M = sg.tile([128, N], FP16)   # rows 0:64 d', rows 64:128 Lk
            # dummy activations FIRST: walrus inserts each ACT_TABLE_LOAD
            # (1283ns, on the ACT engine) right before the first ACTIVATE
            # that needs its table, so warming both tables here runs the
            # loads during the DMA wait instead of delaying the first sqrt.
            ones1 = sg.tile([1, A], FP16)
            nc.vector.memset(ones1[:], 1.0)
            onef = sg.tile([1, 1], F32)
            nc.vector.memset(onef[:], 1.0)
            dmy = sg.tile([1, 1], F32)
            nc.scalar.activation(dmy[:], onef[:], AF.Sqrt, bias=onef[:],
                                 scale=onef[:])
            nc.scalar.activation(dmy[:], onef[:], AF.Relu, bias=onef[:],
                                 scale=-1.0)

            # rept leads the SP queue as one 1152B-per-partition DMA (big
            # descriptors drain fastest), selx/sel follow it there; the ACT
            # queue carries only the tiny sqj/aux2 so its sequencer frees
            # for the dummy activations while the tables load; Lk on pool.
            nc.sync.dma_start(rept_s[:], rept_d[:])
            nc.sync.dma_start(selx_s[:], selx_d[:])
            nc.sync.dma_start(sel_s[:], sel_d[:])
            nc.scalar.dma_start(sqj_s[:], sqj_d[:])
            nc.scalar.dma_start(aux2_s[:], aux2_d[:])
            nc.gpsimd.dma_start(M[NLAB:128, :], lk_d[:])
            pidx_s = aux2_s[:, 0:Tp]
            bias_s = aux2_s[0:A, Tp:Tp + 1]

            # ---- constants / one-offs that overlap the DMA wait
            iota_f = sg.tile([128, N], F32)
            nc.gpsimd.iota(
                iota_f[:], [[1, N]], channel_multiplier=0,
                allow_small_or_imprecise_dtypes=True,
            )


            # ---- d2 accumulation: P = dot - sq_j/2 in column halves, one
            # PSUM bank per half so each bank sees a single clean
            # accumulation group (interleaved start-flags on one bank wipe
            # each other).  aug pass first (its row lands early over
            # SWDGE); h1 stops before h2 so sqrt h1 overlaps PE h2.
            d2a = ppda.tile([A, 256], F32, tag="d2a", name="d2a")
            d2b = ppdb.tile([A, 256], F32, tag="d2b", name="d2b")
            gys = [ppg.tile([128, N], F32, tag="gy", name=f"gy{t}")
                   for t in range(Tp)]
            # d2 groups: one fp8 DoubleRow pass fuses both 128-dim chunks
            # (K=256 packed two-per-partition at 0.5 cycles/row); the aug
            # (sqj) pass comes last so PE never stalls on it
            DR = mybir.MatmulPerfMode.DoubleRow
            nc.tensor.matmul(d2a[:], rept_s[:, :, N:], rept_s[:, :, 0:256],
                             start=True, stop=False, perf_mode=DR,
                             skip_group_check=True)
            nc.tensor.matmul(d2a[:], ones1[:], sqj_s[0:1, 0:256],
                             start=False, stop=True, skip_group_check=True)
            nc.tensor.matmul(d2b[:], rept_s[:, :, N:], rept_s[:, :, 256:512],
                             start=True, stop=False, perf_mode=DR,
                             skip_group_check=True)
            nc.tensor.matmul(d2b[:], ones1[:], sqj_s[0:1, 256:512],
                             start=False, stop=True, skip_group_check=True)

            # d' = sqrt(-2*P + (sq_a + 1)) into M rows 0:64, per half
            nc.scalar.activation(M[0:A, 0:256], d2a[:], AF.Sqrt,
                                 bias=bias_s, scale=-2.0)
            nc.scalar.activation(M[0:A, 256:512], d2b[:], AF.Sqrt,
                                 bias=bias_s, scale=-2.0)

            # ---- gathers.  Each tile's 128-col extraction window rides a
            # redundant matmul into a separate PSUM bank (two windows per
            # bank) so the DVE extraction never waits on a full-width gy.
            # gy0 is split in column halves to start right after sqrt h1.
            SC = sg.tile([128, 2 * Tp], F32)
            relbig = sg.tile([128, Tp, N], BF16)
            cscr = sg.tile([128, N], BF16)
            xscr = sg.tile([128, 2, 128], F32)
            xv = sg.tile([128, Tp], F32)
            winA = ppwa.tile([128, 2, 128], F32, tag="winA")
            winB = ppwb.tile([128, 2, 128], F32, tag="winB")
            wins = [winA[:, 0, :], winA[:, 1, :], winB[:, 0, :], winB[:, 1, :]]

            def selt(t):
                return sel_s[:, t * 128:(t + 1) * 128]

            def selxt(t):
                return selx_s[:, t * 128:(t + 1) * 128]

            # the windows use selX (label rows scaled to B+XOFF = margin),
            # so the extraction yields xp = d_ap + margin directly
            nc.tensor.matmul(wins[0], selxt(0), M[:, 0:128],
                             start=True, stop=True)
            nc.tensor.matmul(gys[0][:, 0:256], selt(0), M[:, 0:256],
                             start=True, stop=True)
            nc.tensor.matmul(wins[1], selxt(1), M[:, 128:256],
                             start=True, stop=True)
            nc.tensor.matmul(gys[0][:, 256:512], selt(0), M[:, 256:512],
                             start=True, stop=True)
            nc.tensor.matmul(wins[2], selxt(2), M[:, 256:384],
                             start=True, stop=True)
            nc.tensor.matmul(wins[3], selxt(3), M[:, 384:512],
                             start=True, stop=True)
            for t in range(1, Tp):
                nc.tensor.matmul(gys[t][:], selt(t), M[:],
                                 start=True, stop=True)

            # ---- extraction (DVE): xv[p] = d_ap + margin from the window
            def extract(t):
                w0 = t * 128
                nc.vector.scalar_tensor_tensor(
                    out=xscr[:, t % 2, :], in0=iota_f[:, w0:w0 + 128],
                    scalar=pidx_s[:, t:t + 1], in1=wins[t],
                    op0=OP.is_equal, op1=OP.mult, accum_out=xv[:, t:t + 1],
                )

            for t in range(Tp):
                extract(t)

            # ---- relu + S on ACT; counts on DVE from the bf16 relu output
            for t in range(Tp):
                nc.scalar.activation(
                    relbig[:, t, :], gys[t][:], AF.Relu, bias=xv[:, t:t + 1],
                    scale=-1.0, accum_out=SC[:, t:t + 1],
                )
            for t in range(Tp):
                nc.vector.tensor_scalar(
                    cscr[:], relbig[:, t, :], 0.0, 0.0,
                    OP.is_gt, OP.add, accum_out=SC[:, Tp + t:Tp + t + 1],
                )

            # pool's stream must outlive its own SWDGE input DMA (Lk): a
            # tiny read makes the tile tracker emit the wait.
            pguard = sg.tile([1, 8], FP16)
            nc.gpsimd.tensor_copy(pguard[0:1, 0:8], M[NLAB:NLAB + 1, 0:8])

    # Output DMA emitted AFTER the tile exit: SP executes it post-drain
    # (every compute tick already waited out, so SC is final) and nothing
    # waits on its completion - it flies during the runtime's semaphore
    # sweep.  The completion sem is required plumbing; nothing waits on it.
    out_insts = [nc.sync.dma_start(out_d[:], SC[:]).then_inc(out_sem, 16)]
    if dbg:
        out_insts.append(
            nc.sync.dma_start(dbgM_d[:], M[:]).then_inc(out_sem, 16))
        xvxp = sg_xvxp = None
        out_insts.append(
            nc.sync.dma_start(dbgx_d[:, 0:Tp], xv[:]).then_inc(out_sem, 16))

    def _concrete(arg):
        t = arg.bass_ap.tensor
        if hasattr(t, "concrete_tensor"):
            try:
                arg.bass_ap.tensor = t.concrete_tensor()
            except Exception:
                pass
        return arg.bass_ap

    for oi in out_insts:
        raw = oi.ins
        raw.ins, raw.outs = nc.sync.lower_symbolic_args(
            raw.ins, raw.outs, _concrete, raw.debug)

    nc.finalize()
    return nc


def _prep(rep: np.ndarray, labels: np.ndarray):
    """Host-side prep: shard anchors, enumerate pairs, build layouts."""
    rep = np.ascontiguousarray(np.asarray(rep, dtype=np.float32))
    labels = np.asarray(labels).astype(np.int64)
    cnt = np.bincount(labels, minlength=NLAB)
    ppa = cnt[labels] - 1              # pairs per anchor
    rows_of = [np.nonzero(labels == l)[0] for l in range(NLAB)]

    # balance pair counts across 8 groups of exactly 64 anchors
    order = np.argsort(-ppa, kind="stable")
    groups = [[] for _ in range(NCORES)]
    loads = [0] * NCORES
    for a in order:
        cand = min((i for i in range(NCORES) if len(groups[i]) < A),
                   key=lambda j: loads[j])
        groups[cand].append(int(a))
        loads[cand] += int(ppa[a])
    Tp = max(1, (max(loads) + 127) // 128)

    rep8 = rep.astype(ml_dtypes.float8_e4m3fn)
    sqj_all = (rep8.astype(np.float32) ** 2).sum(axis=1)   # from quantized rep
    in_maps = []
    for c in range(NCORES):
        anchors = groups[c]
        # bin-pack the 512 columns into Tp blocks of 128 so each block's
        # positive-pair count fits one 128-pair tile
        anchset = np.zeros(N, bool)
        anchset[anchors] = True
        m = np.zeros(N, np.int64)
        for p in range(N):
            m[p] = sum(1 for a in rows_of[labels[p]]
                       if anchset[a] and a != p)
        blocks = [[] for _ in range(Tp)]
        bsum = [0] * Tp
        for p in np.argsort(-m, kind="stable"):
            cand = [b for b in range(Tp)
                    if len(blocks[b]) < 128 and bsum[b] + m[p] <= 128]
            b = min(cand, key=lambda x: bsum[x])
            blocks[b].append(int(p))
            bsum[b] += int(m[p])
        perm = np.array([p for b in blocks for p in b], np.int64)
        colof = np.empty(N, np.int64)
        colof[perm] = np.arange(N)
        aidx = {int(a): i for i, a in enumerate(anchors)}

        repp = rep8[perm]                                  # [512, 256]
        repe = np.concatenate([repp, rep8[anchors]])       # + anchor dup
        rept = np.ascontiguousarray(
            repe.T.reshape(2, 128, N + A).transpose(1, 0, 2)
        )
        sel = np.zeros((128, Tp * 128), np.float16)
        lk = np.ascontiguousarray(
            (labels[perm][None, :] == np.arange(NLAB)[:, None])
            .astype(np.float16))
        sqj = np.ascontiguousarray(
            (-0.5 * sqj_all[perm])[None, :].astype(np.float16))
        aux2 = np.zeros((128, Tp + 1), np.float32)
        aux2[0:A, Tp] = sqj_all[anchors] + 1.0
        for t in range(Tp):
            aux2[:, t] = 128 * t    # pad pidx inside window -> xv = 0
        nt = [0] * Tp
        for ga in anchors:
            a = aidx[ga]
            for p in rows_of[labels[ga]]:
                if p == ga:
                    continue
                t, r = colof[p] // 128, nt[colof[p] // 128]
                nt[t] += 1
                sel[a, t * 128 + r] = 1.0
                sel[A + labels[ga], t * 128 + r] = BIGB
                aux2[r, t] = colof[p]
        selx = sel.copy()
        selx[A:2 * A] *= (BIGB + XOFF) / BIGB   # label rows -> margin
        in_maps.append({"rept": rept, "sel": sel, "selx": selx, "lk": lk,
                        "sqj": sqj, "aux2": aux2})
    return Tp, in_maps


def _run(rep, labels, trace=False):
    Tp, in_maps = _prep(rep, labels)
    if Tp not in _cache:
        _cache[Tp] = _build(Tp)
    nc = _cache[Tp]
    res = run_bass_kernel_spmd(nc, in_maps, list(range(NCORES)), trace=trace)
    outs = np.stack([res.results[c]["out"] for c in range(NCORES)])  # [8,128,2Tp]
    S = float(outs[:, :, :Tp].sum(dtype=np.float64))
    C = float(outs[:, :, Tp:].sum(dtype=np.float64))
    loss = np.float32(S / (C + EPS))
    return np.asarray(loss, dtype=np.float32), res


def kernel(rep, labels):
    loss, _ = _run(rep, labels, trace=False)
    return loss
